# revision 2
# baseline (speedup 1.0000x reference)
"""BWGNN (Bernstein-polynomial graph conv, D=2) on 8 Trainium2 NeuronCores.

Key algebra: inside each polyconv the feat sequence f0, f1=f0-A f0, f2=f1-A f1
is theta-independent, so the device needs only TWO SpMMs (not six), and the
Bernstein mixing folds into W3 on the host:
    out = relu([f0|f1|f2] @ W3' + b3) @ W4 + b4,
    W3'[k*H+i, j] = sum_t theta[t][k] * W3[t*H+i, j].

Distribution (8 cores, SPMD single program):
- nodes row-sharded: core c owns rows [c*12500, (c+1)*12500), padded to 12544.
- dense layers stream feature-major fp16 with stationary weights.
- SpMM: edges partitioned by destination; per (128-row dest block b,
  source-column quarter q) cell the edges are padded to whole chunks of 128
  (chunk counts equalized across cores so one program serves all).  Per
  chunk: Ant dma_gather fetches the 128 source rows (fp32, 256B each) from an
  AllGathered [100352, 64] HBM table (int16 gather indices are quarter-local,
  hence the 4 quarters), a single dual-op DVE tensor_scalar builds the
  one-hot-times-value matrix S[e, r] = vals[e] * (rowloc[e] == r), and the PE
  accumulates S.T @ G into the block's PSUM tile.  feat_next = feat - PSUM.
- cross-core neighbor features: fp32 AllGather (replica group = all 8 cores).
"""
import math
import numpy as np

import concourse.bass as bass
import concourse.bacc as bacc
import concourse.mybir as mybir
from concourse.tile import TileContext
from concourse.masks import make_identity
from concourse import bass_utils

N = 100000
F_IN = 128
H = 64
NCLS = 2
D = 2
W = 8                   # cores
R = 12500               # real rows per core
RP = 12544              # padded rows per core (98 * 128)
NB = RP // 128          # 98 dest blocks per core
NPAD = W * RP           # 100352 padded table rows
NQ = 4                  # source-column quarters (int16 index range)
QS = NPAD // NQ         # 25088 rows per quarter
GRP = 8                 # dest blocks per gather/S group
F16 = mybir.dt.float16
F32 = mybir.dt.float32
I16 = mybir.dt.int16


def _theta2():
    P = np.polynomial.polynomial
    thetas = []
    for i in range(D + 1):
        beta = math.factorial(i) * math.factorial(D - i) / math.factorial(D + 1)
        c = P.polymul(P.polypow([0.0, 0.5], i), P.polypow([1.0, -0.5], D - i)) / beta
        c = np.pad(c, (0, D + 1 - len(c)))
        thetas.append(c.astype(np.float64))
    return thetas


def _prep_edges(adj_rows, adj_cols, adj_vals):
    """Partition edges by (core, dest block, source quarter); pad each cell to
    whole 128-edge chunks with counts equalized across cores.

    Returns:
      idx_wrapped[c]: [128, 8*T] int16 gather indices (16-wrapped, 8x replicated)
      rowv[c], vals[c]: [128, T] fp16 per-slot dest-row-in-block / edge weight
      cells: [NB, NQ] chunks per cell (shared schedule)
      tq: [NQ] chunks per quarter
      q_of_chunk, qidx_of_chunk: global chunk t -> (quarter, index in quarter)
    """
    core = adj_rows // R
    rloc = adj_rows - core * R
    blk = rloc // 128
    rowin = rloc % 128
    colp = (adj_cols // R) * RP + (adj_cols % R)     # padded table row
    q = colp // QS
    qoff = colp - q * QS

    counts = np.zeros((W, NB, NQ), dtype=np.int64)
    np.add.at(counts, (core, blk, q), 1)
    cells = np.maximum(np.ceil(counts.max(axis=0) / 128.0).astype(np.int64), 1)

    tq = cells.sum(axis=0)
    T = int(tq.sum())
    q_of_chunk = np.zeros(T, dtype=np.int64)
    qidx_of_chunk = np.zeros(T, dtype=np.int64)
    qpos = np.zeros(NQ, dtype=np.int64)
    cell_chunk_start = np.zeros((NB, NQ), dtype=np.int64)
    t = 0
    for b in range(NB):
        for qq in range(NQ):
            cell_chunk_start[b, qq] = qpos[qq]
            for _ in range(cells[b, qq]):
                q_of_chunk[t] = qq
                qidx_of_chunk[t] = qpos[qq]
                qpos[qq] += 1
                t += 1
    t_of_q_qidx = np.zeros((NQ, int(tq.max())), dtype=np.int64)
    for tt in range(T):
        t_of_q_qidx[q_of_chunk[tt], qidx_of_chunk[tt]] = tt

    order = np.lexsort((q, blk, core))
    sc, sb, sq = core[order], blk[order], q[order]
    s_qoff, s_rowin, s_val = qoff[order], rowin[order], adj_vals[order]

    smat = np.zeros((W, 128, T * 128), dtype=np.float16)
    idx16 = [[np.zeros(int(tq[qq]) * 128, dtype=np.int16) for qq in range(NQ)]
             for _ in range(W)]

    csel = np.searchsorted(sc, np.arange(W + 1))
    for c in range(W):
        lo, hi = csel[c], csel[c + 1]
        b_arr, q_arr = sb[lo:hi], sq[lo:hi]
        qo, ri, vv = s_qoff[lo:hi], s_rowin[lo:hi], s_val[lo:hi]
        m = len(b_arr)
        if not m:
            continue
        key = b_arr * NQ + q_arr
        brk = np.nonzero(np.diff(key))[0] + 1
        starts = np.concatenate([[0], brk])
        lens = np.diff(np.concatenate([starts, [m]]))
        pos = np.arange(m) - np.repeat(starts, lens)
        slot_in_q = cell_chunk_start[b_arr, q_arr] * 128 + pos
        for qq in range(NQ):
            sel = q_arr == qq
            idx16[c][qq][slot_in_q[sel]] = qo[sel].astype(np.int16)
        kchunk = pos // 128
        p = pos % 128
        tglob = t_of_q_qidx[q_arr, cell_chunk_start[b_arr, q_arr] + kchunk]
        smat[c, p, tglob * 128 + ri] = vv.astype(np.float16)

    idx_wrapped = []
    for c in range(W):
        parts = []
        for qq in range(NQ):
            a = idx16[c][qq]
            parts.append(a.reshape(len(a) // 16, 16).T)
        cat = np.concatenate(parts, axis=1)
        idx_wrapped.append(np.tile(cat, (8, 1)).copy())
    return idx_wrapped, smat, cells, tq, q_of_chunk, qidx_of_chunk


def _build(cells, tq, q_of_chunk, qidx_of_chunk):
    T = int(tq.sum())
    qbase = [8 * int(tq[:qq].sum()) for qq in range(NQ)]
    # group schedule: chunks per group (block-major global order)
    groups = []
    t0 = 0
    for g0 in range(0, NB, GRP):
        blocks = list(range(g0, min(g0 + GRP, NB)))
        tcount = int(sum(cells[b, qq] for b in blocks for qq in range(NQ)))
        groups.append((blocks, t0, tcount))
        t0 += tcount
    gmax = max(tc for _, _, tc in groups)

    nc = bacc.Bacc("TRN2")
    rg = [list(range(W))]

    xT = nc.dram_tensor("xT", [F_IN, RP], F16, kind="ExternalInput")
    w1 = nc.dram_tensor("w1", [F_IN, H], F16, kind="ExternalInput")
    w2 = nc.dram_tensor("w2", [H, H], F16, kind="ExternalInput")
    w3 = nc.dram_tensor("w3", [3 * H, H], F16, kind="ExternalInput")
    w4 = nc.dram_tensor("w4", [H, NCLS], F16, kind="ExternalInput")
    b1 = nc.dram_tensor("b1", [H, 1], F32, kind="ExternalInput")
    b2 = nc.dram_tensor("b2", [H, 1], F32, kind="ExternalInput")
    b3 = nc.dram_tensor("b3", [H, 1], F32, kind="ExternalInput")
    b4 = nc.dram_tensor("b4", [NCLS, 1], F32, kind="ExternalInput")
    idx_t = nc.dram_tensor("idx", [128, T * 8], I16, kind="ExternalInput")
    smat_t = nc.dram_tensor("smat", [128, T * 128], F16, kind="ExternalInput")
    out_t = nc.dram_tensor("out", [NCLS, RP], F32, kind="ExternalOutput")

    ag_in = [nc.dram_tensor(f"agin{i}", [RP, H], F32, kind="Internal")
             for i in range(2)]
    ag_out = [nc.dram_tensor(f"agout{i}", [NPAD, H], F32, kind="Internal",
                             addr_space="Shared") for i in range(2)]

    PCH = 448            # dense-layer column chunk (28 * 448 = 12544)

    with TileContext(nc) as tc:
        with tc.tile_pool(name="c0", bufs=1) as cpool, \
             tc.tile_pool(name="mm", bufs=3) as mpool, \
             tc.tile_pool(name="gg", bufs=2) as gpool, \
             tc.tile_pool(name="ss", bufs=4) as spool, \
             tc.tile_pool(name="ps", bufs=2, space="PSUM") as pspool, \
             tc.tile_pool(name="pb", bufs=2, space="PSUM") as pbpool:

            ident = cpool.tile([128, 128], F16)
            make_identity(nc, ident[:])

            def load_const(name, src, shape, dt):
                tile = cpool.tile(shape, dt, tag=name)
                nc.sync.dma_start(out=tile[:], in_=src)
                return tile

            w1_sb = load_const("w1", w1[:], [F_IN, H], F16)
            w2_sb = load_const("w2", w2[:], [H, H], F16)
            w3ab_sb = load_const("w3ab", w3[0:128, :], [128, H], F16)
            w3c_sb = load_const("w3c", w3[128:192, :], [H, H], F16)
            w4_sb = load_const("w4", w4[:], [H, NCLS], F16)
            b1_sb = load_const("b1", b1[:], [H, 1], F32)
            b2_sb = load_const("b2", b2[:], [H, 1], F32)
            b3_sb = load_const("b3", b3[:], [H, 1], F32)
            b4_sb = load_const("b4", b4[:], [NCLS, 1], F32)
            idx_sb = load_const("idx", idx_t[:], [128, T * 8], I16)

            h1_f2 = cpool.tile([128, RP], F16)   # h1 then feat2 (fm) on p0..63
            h_cat = cpool.tile([128, RP], F16)   # feat0 on p0..63, feat1 on p64..127
            f0_rm = cpool.tile([128, NB * H], F16)
            f1_rm = cpool.tile([128, NB * H], F16)
            f2_rm = f0_rm     # feat0_rm is dead once SpMM1's subtract ran

            # ---------- MLP1 + MLP2 (feature-major fp16) ----------
            for o in range(0, RP, PCH):
                xt = mpool.tile([F_IN, PCH], F16, tag="xin")
                nc.sync.dma_start(out=xt[:], in_=xT[:, o:o + PCH])
                pt = pspool.tile([H, PCH], F32, tag="pmlp", space="PSUM")
                nc.tensor.matmul(pt[:], lhsT=w1_sb[:], rhs=xt[:],
                                 start=True, stop=True)
                nc.scalar.activation(h1_f2[0:H, o:o + PCH], pt[:],
                                     mybir.ActivationFunctionType.Relu,
                                     bias=b1_sb[:], scale=1.0)
            for o in range(0, RP, PCH):
                pt = pspool.tile([H, PCH], F32, tag="pmlp", space="PSUM")
                nc.tensor.matmul(pt[:], lhsT=w2_sb[:], rhs=h1_f2[0:H, o:o + PCH],
                                 start=True, stop=True)
                nc.scalar.activation(h_cat[0:H, o:o + PCH], pt[:],
                                     mybir.ActivationFunctionType.Relu,
                                     bias=b2_sb[:], scale=1.0)

            # ---------- feat0 -> row-major, ship to AllGather ----------
            for b in range(NB):
                pt = pbpool.tile([128, 128], F16, tag="ptr", space="PSUM")
                nc.tensor.transpose(pt[0:128, 0:H],
                                    h_cat[0:H, b * 128:(b + 1) * 128],
                                    ident[0:H, 0:H])
                nc.vector.tensor_copy(f0_rm[:, b * H:(b + 1) * H], pt[0:128, 0:H])
            nc.gpsimd.dma_start(
                out=ag_in[0][:].rearrange("(t p) h -> p t h", p=128),
                in_=f0_rm[:].rearrange("p (t h) -> p t h", h=H))
            nc.gpsimd.collective_compute(
                "AllGather", mybir.AluOpType.bypass, replica_groups=rg,
                ins=[ag_in[0][:]], outs=[ag_out[0][:]])

            # ---------- SpMM pass ----------
            def spmm(src, cur_rm, nxt_rm, ag_next):
                src_flat = src[:].rearrange("n h -> (n h)")
                qviews = [src_flat[qq * QS * H:(qq + 1) * QS * H].rearrange(
                    "(q s) -> q s", s=H) for qq in range(NQ)]
                cast_flip = [0]
                for blocks, t0, tcount in groups:
                    gt = gpool.tile([128, gmax * H], F32, tag="gbuf")
                    sbuf_s = gpool.tile([128, gmax * 128], F16, tag="sbuf_s")
                    nc.sync.dma_start(
                        out=sbuf_s[:, :tcount * 128],
                        in_=smat_t[:, t0 * 128:(t0 + tcount) * 128])
                    goff = 0
                    gslot = {}
                    for qq in range(NQ):
                        ks = [t for t in range(t0, t0 + tcount)
                              if q_of_chunk[t] == qq]
                        if not ks:
                            continue
                        # contiguous qidx run within the group
                        lo = int(qidx_of_chunk[ks[0]])
                        assert int(qidx_of_chunk[ks[-1]]) == lo + len(ks) - 1
                        for s0 in range(0, len(ks), 64):
                            nch = min(64, len(ks) - s0)
                            nc.gpsimd.dma_gather(
                                out_ap=gt[:, goff * H:(goff + nch) * H]
                                .rearrange("p (t e) -> p t e", e=H),
                                in_ap=qviews[qq],
                                idxs_ap=idx_sb[:, qbase[qq] + 8 * (lo + s0):
                                               qbase[qq] + 8 * (lo + s0 + nch)],
                                num_idxs=nch * 128,
                                num_idxs_reg=nch * 128,
                                elem_size=H,
                                single_packet=False,
                            )
                            for j in range(nch):
                                gslot[ks[s0 + j]] = goff + j
                            goff += nch
                    t = t0
                    for b in blocks:
                        pt = pbpool.tile([128, H], F32, tag="pblk", space="PSUM")
                        nchunks = int(sum(cells[b, qq] for qq in range(NQ)))
                        for k in range(nchunks):
                            gsl = gslot[t]
                            g16 = spool.tile([128, H], F16, tag="g16")
                            nc.scalar.copy(
                                g16[:], gt[:, gsl * H:(gsl + 1) * H])
                            nc.tensor.matmul(
                                pt[:],
                                lhsT=sbuf_s[:, (t - t0) * 128:(t - t0 + 1) * 128],
                                rhs=g16[:],
                                start=(k == 0), stop=(k == nchunks - 1))
                            t += 1
                        nc.vector.tensor_tensor(
                            out=nxt_rm[:, b * H:(b + 1) * H],
                            in0=cur_rm[:, b * H:(b + 1) * H], in1=pt[:],
                            op=mybir.AluOpType.subtract)
                if ag_next is not None:
                    nc.gpsimd.dma_start(
                        out=ag_next[:].rearrange("(t p) h -> p t h", p=128),
                        in_=nxt_rm[:].rearrange("p (t h) -> p t h", h=H))

            spmm(ag_out[0], f0_rm, f1_rm, ag_in[1])
            nc.gpsimd.collective_compute(
                "AllGather", mybir.AluOpType.bypass, replica_groups=rg,
                ins=[ag_in[1][:]], outs=[ag_out[1][:]])
            spmm(ag_out[1], f1_rm, f2_rm, None)

            # ---------- feat1/feat2 back to feature-major ----------
            for b in range(NB):
                pt = pbpool.tile([128, 128], F16, tag="ptr", space="PSUM")
                nc.tensor.transpose(pt[0:H, 0:128], f1_rm[:, b * H:(b + 1) * H],
                                    ident[:])
                nc.vector.tensor_copy(h_cat[H:128, b * 128:(b + 1) * 128],
                                      pt[0:H, 0:128])
            for b in range(NB):
                pt = pbpool.tile([128, 128], F16, tag="ptr", space="PSUM")
                nc.tensor.transpose(pt[0:H, 0:128], f2_rm[:, b * H:(b + 1) * H],
                                    ident[:])
                nc.vector.tensor_copy(h1_f2[0:H, b * 128:(b + 1) * 128],
                                      pt[0:H, 0:128])

            # ---------- MLP3 + MLP4 fused ----------
            for o in range(0, RP, PCH):
                pt = pspool.tile([H, PCH], F32, tag="pmlp", space="PSUM")
                nc.tensor.matmul(pt[:], lhsT=w3ab_sb[:], rhs=h_cat[:, o:o + PCH],
                                 start=True, stop=False)
                nc.tensor.matmul(pt[:], lhsT=w3c_sb[:], rhs=h1_f2[0:H, o:o + PCH],
                                 start=False, stop=True)
                h3 = mpool.tile([H, PCH], F16, tag="h3")
                nc.scalar.activation(h3[:], pt[:],
                                     mybir.ActivationFunctionType.Relu,
                                     bias=b3_sb[:], scale=1.0)
                po = pspool.tile([NCLS, PCH], F32, tag="pout", space="PSUM")
                nc.tensor.matmul(po[:], lhsT=w4_sb[:], rhs=h3[:],
                                 start=True, stop=True)
                ot = mpool.tile([NCLS, PCH], F32, tag="ot")
                nc.scalar.activation(ot[:], po[:],
                                     mybir.ActivationFunctionType.Identity,
                                     bias=b4_sb[:], scale=1.0)
                nc.sync.dma_start(out=out_t[:, o:o + PCH], in_=ot[:])

    nc.compile()
    return nc


def _plan(in_feat, adj_rows, adj_cols, adj_vals, W1, b1, W2, b2, W3, b3, W4, b4):
    in_feat = np.asarray(in_feat, dtype=np.float32)
    adj_rows = np.asarray(adj_rows).astype(np.int64)
    adj_cols = np.asarray(adj_cols).astype(np.int64)
    adj_vals = np.asarray(adj_vals, dtype=np.float32)

    thetas = _theta2()
    W3 = np.asarray(W3, dtype=np.float64)
    W3p = np.zeros((3 * H, H), dtype=np.float64)
    for k in range(D + 1):
        for t in range(D + 1):
            W3p[k * H:(k + 1) * H] += thetas[t][k] * W3[t * H:(t + 1) * H]

    (idx_wrapped, smat, cells, tq, q_of_chunk, qidx_of_chunk
     ) = _prep_edges(adj_rows, adj_cols, adj_vals)

    nc = _build(cells, tq, q_of_chunk, qidx_of_chunk)

    in_maps = []
    for c in range(W):
        shard = np.zeros((F_IN, RP), dtype=np.float16)
        shard[:, :R] = in_feat[c * R:(c + 1) * R].T.astype(np.float16)
        in_maps.append({
            "xT": shard,
            "w1": np.asarray(W1).astype(np.float16),
            "w2": np.asarray(W2).astype(np.float16),
            "w3": W3p.astype(np.float16),
            "w4": np.asarray(W4).astype(np.float16),
            "b1": np.asarray(b1, dtype=np.float32).reshape(H, 1),
            "b2": np.asarray(b2, dtype=np.float32).reshape(H, 1),
            "b3": np.asarray(b3, dtype=np.float32).reshape(H, 1),
            "b4": np.asarray(b4, dtype=np.float32).reshape(NCLS, 1),
            "idx": idx_wrapped[c],
            "smat": smat[c],
        })
    return nc, in_maps


def kernel(in_feat, adj_rows, adj_cols, adj_vals, W1, b1, W2, b2, W3, b3, W4, b4):
    nc, in_maps = _plan(in_feat, adj_rows, adj_cols, adj_vals,
                        W1, b1, W2, b2, W3, b3, W4, b4)
    res = bass_utils.run_bass_kernel_spmd(nc, in_maps, list(range(W)))
    out = np.concatenate(
        [res.results[c]["out"][:, :R].T for c in range(W)], axis=0)
    return np.ascontiguousarray(out, dtype=np.float32)



# revision 7
# speedup vs baseline: 1.5241x; 1.5241x over previous
"""BWGNN (Bernstein-polynomial graph conv, D=2) on 8 Trainium2 NeuronCores.

Key algebra: inside each polyconv the feat sequence f0, f1=f0-A f0, f2=f1-A f1
is theta-independent, so the device needs only TWO SpMMs (not six), and the
Bernstein mixing folds into W3 on the host:
    out = relu([f0|f1|f2] @ W3' + b3) @ W4 + b4,
    W3'[k*H+i, j] = sum_t theta[t][k] * W3[t*H+i, j].

Distribution (8 cores, SPMD single program):
- nodes row-sharded: core c owns rows [c*12500, (c+1)*12500), padded to 12544.
- dense layers stream feature-major fp16 with stationary weights.
- SpMM: edges partitioned by destination; per (128-row dest block b,
  source-column quarter q) cell the edges are padded to whole chunks of 128
  (chunk counts equalized across cores so one program serves all).  Per
  chunk: dma_gather fetches the 128 source rows (fp32, 256B each) from an
  AllGathered fp32 HBM table (int16 gather indices are quarter-local, hence
  the 4 quarters); a dual-op DVE tensor_scalar builds the one-hot-times-value
  matrix S[e, r] = vals[e] * (rowv[e] == r) from an iota const, and the PE
  accumulates S.T @ G16 into the block's PSUM tile.  feat_next = feat - PSUM.
- gathers round-robin over 4 SWDGE queues; gathered fp32 is converted to fp16
  in one batched Act copy per gather (not per chunk).
- the AllGather table is stored in SBUF-dump order (partition-major), so the
  table write is one contiguous DMA; gather indices are host-permuted.
"""
import math
import numpy as np

import concourse.bass as bass
import concourse.bacc as bacc
import concourse.mybir as mybir
from concourse.tile import TileContext
from concourse.masks import make_identity
from concourse import bass_utils

N = 100000
F_IN = 128
H = 64
NCLS = 2
D = 2
W = 8                   # cores
R = 12500               # real rows per core
RP = 12544              # padded rows per core (98 * 128)
NB = RP // 128          # 98 dest blocks per core
NPAD = W * RP           # 100352 padded table rows
NQ = 4                  # source-column quarters (int16 index range)
QS = NPAD // NQ         # 25088 rows per quarter
GRP = 5                 # dest blocks per gather/S group
F16 = mybir.dt.float16
F32 = mybir.dt.float32
I16 = mybir.dt.int16


def _theta2():
    P = np.polynomial.polynomial
    thetas = []
    for i in range(D + 1):
        beta = math.factorial(i) * math.factorial(D - i) / math.factorial(D + 1)
        c = P.polymul(P.polypow([0.0, 0.5], i), P.polypow([1.0, -0.5], D - i)) / beta
        c = np.pad(c, (0, D + 1 - len(c)))
        thetas.append(c.astype(np.float64))
    return thetas


def _prep_edges(adj_rows, adj_cols, adj_vals):
    """Partition edges by (core, dest block, source quarter); pad each cell to
    whole 128-edge chunks with counts equalized across cores.

    Table rows live in SBUF-dump order: node (c, rl) -> row
    c*RP/... = c*12544 + (rl%128)*NB + rl//128 (in units of H-element rows).

    Returns:
      idx_wrapped[c]: [128, 8*T] int16 gather indices (16-wrapped, 8x replicated)
      rowv[c], vals[c]: [128, T] fp16 per-slot dest-row-in-block / edge weight
      cells: [NB, NQ] chunks per cell (shared schedule)
      tq: [NQ] chunks per quarter
      q_of_chunk, qidx_of_chunk: global chunk t -> (quarter, index in quarter)
    """
    core = adj_rows // R
    rloc = adj_rows - core * R
    blk = rloc // 128
    rowin = rloc % 128
    csrc = adj_cols // R
    rsrc = adj_cols - csrc * R
    colp = csrc * RP + (rsrc % 128) * NB + rsrc // 128   # permuted table row
    q = colp // QS
    qoff = colp - q * QS

    counts = np.zeros((W, NB, NQ), dtype=np.int64)
    np.add.at(counts, (core, blk, q), 1)
    cells = np.maximum(np.ceil(counts.max(axis=0) / 128.0).astype(np.int64), 1)

    tq = cells.sum(axis=0)
    T = int(tq.sum())
    q_of_chunk = np.zeros(T, dtype=np.int64)
    qidx_of_chunk = np.zeros(T, dtype=np.int64)
    qpos = np.zeros(NQ, dtype=np.int64)
    cell_chunk_start = np.zeros((NB, NQ), dtype=np.int64)
    t = 0
    for b in range(NB):
        for qq in range(NQ):
            cell_chunk_start[b, qq] = qpos[qq]
            for _ in range(cells[b, qq]):
                q_of_chunk[t] = qq
                qidx_of_chunk[t] = qpos[qq]
                qpos[qq] += 1
                t += 1

    order = np.lexsort((q, blk, core))
    sc, sb, sq = core[order], blk[order], q[order]
    s_qoff, s_rowin, s_val = qoff[order], rowin[order], adj_vals[order]

    rowv = np.zeros((W, 128, T), dtype=np.float32)
    vals = np.zeros((W, 128, T), dtype=np.float32)
    idx16 = [[np.zeros(int(tq[qq]) * 128, dtype=np.int16) for qq in range(NQ)]
             for _ in range(W)]

    t_of_q_qidx = np.zeros((NQ, int(tq.max())), dtype=np.int64)
    for tt in range(T):
        t_of_q_qidx[q_of_chunk[tt], qidx_of_chunk[tt]] = tt

    csel = np.searchsorted(sc, np.arange(W + 1))
    for c in range(W):
        lo, hi = csel[c], csel[c + 1]
        b_arr, q_arr = sb[lo:hi], sq[lo:hi]
        qo, ri, vv = s_qoff[lo:hi], s_rowin[lo:hi], s_val[lo:hi]
        m = len(b_arr)
        if not m:
            continue
        key = b_arr * NQ + q_arr
        brk = np.nonzero(np.diff(key))[0] + 1
        starts = np.concatenate([[0], brk])
        lens = np.diff(np.concatenate([starts, [m]]))
        pos = np.arange(m) - np.repeat(starts, lens)
        slot_in_q = cell_chunk_start[b_arr, q_arr] * 128 + pos
        for qq in range(NQ):
            sel = q_arr == qq
            idx16[c][qq][slot_in_q[sel]] = qo[sel].astype(np.int16)
        kchunk = pos // 128
        p = pos % 128
        tglob = t_of_q_qidx[q_arr, cell_chunk_start[b_arr, q_arr] + kchunk]
        rowv[c, p, tglob] = ri.astype(np.float32)
        vals[c, p, tglob] = vv.astype(np.float32)

    idx_wrapped = []
    for c in range(W):
        parts = []
        for qq in range(NQ):
            a = idx16[c][qq]
            parts.append(a.reshape(len(a) // 16, 16).T)
        cat = np.concatenate(parts, axis=1)
        idx_wrapped.append(np.tile(cat, (8, 1)).copy())
    return idx_wrapped, rowv, vals, cells, tq, q_of_chunk, qidx_of_chunk


def _build(cells, tq, q_of_chunk, qidx_of_chunk):
    T = int(tq.sum())
    qbase = [8 * int(tq[:qq].sum()) for qq in range(NQ)]
    # group schedule: chunks per group (block-major global order)
    groups = []
    t0 = 0
    for g0 in range(0, NB, GRP):
        blocks = list(range(g0, min(g0 + GRP, NB)))
        tcount = int(sum(cells[b, qq] for b in blocks for qq in range(NQ)))
        groups.append((blocks, t0, tcount))
        t0 += tcount
    gmax = max(tc for _, _, tc in groups)

    nc = bacc.Bacc("TRN2", num_swdge_queues=4)
    rg = [list(range(W))]

    xT = nc.dram_tensor("xT", [F_IN, RP], F16, kind="ExternalInput")
    w1 = nc.dram_tensor("w1", [F_IN, H], F16, kind="ExternalInput")
    w2 = nc.dram_tensor("w2", [H, H], F16, kind="ExternalInput")
    w3 = nc.dram_tensor("w3", [3 * H, H], F16, kind="ExternalInput")
    w4 = nc.dram_tensor("w4", [H, NCLS], F16, kind="ExternalInput")
    b1 = nc.dram_tensor("b1", [H, 1], F32, kind="ExternalInput")
    b2 = nc.dram_tensor("b2", [H, 1], F32, kind="ExternalInput")
    b3 = nc.dram_tensor("b3", [H, 1], F32, kind="ExternalInput")
    b4 = nc.dram_tensor("b4", [NCLS, 1], F32, kind="ExternalInput")
    idx_t = nc.dram_tensor("idx", [128, T * 8], I16, kind="ExternalInput")
    rowv_t = nc.dram_tensor("rowv", [128, T], F32, kind="ExternalInput")
    vals_t = nc.dram_tensor("vals", [128, T], F32, kind="ExternalInput")
    iota_t = nc.dram_tensor("iota", [128, 128], F16, kind="ExternalInput")
    out_t = nc.dram_tensor("out", [NCLS, RP], F32, kind="ExternalOutput")

    ag_in = [nc.dram_tensor(f"agin{i}", [128, NB * H], F32, kind="Internal")
             for i in range(2)]
    ag_out = [nc.dram_tensor(f"agout{i}", [W * 128, NB * H], F32,
                             kind="Internal", addr_space="Shared")
              for i in range(2)]

    PCH = 448            # dense-layer column chunk (28 * 448 = 12544)

    with TileContext(nc) as tc:
        with tc.tile_pool(name="c0", bufs=1) as cpool, \
             tc.tile_pool(name="mm", bufs=3) as mpool, \
             tc.tile_pool(name="gg", bufs=2) as gpool, \
             tc.tile_pool(name="ss", bufs=4) as spool, \
             tc.tile_pool(name="ps", bufs=2, space="PSUM") as pspool, \
             tc.tile_pool(name="pb", bufs=2, space="PSUM") as pbpool:

            ident = cpool.tile([128, 128], F16)
            make_identity(nc, ident[:])

            def load_const(name, src, shape, dt):
                tile = cpool.tile(shape, dt, tag=name)
                nc.sync.dma_start(out=tile[:], in_=src)
                return tile

            w1_sb = load_const("w1", w1[:], [F_IN, H], F16)
            w2_sb = load_const("w2", w2[:], [H, H], F16)
            w3ab_sb = load_const("w3ab", w3[0:128, :], [128, H], F16)
            w3c_sb = load_const("w3c", w3[128:192, :], [H, H], F16)
            w4_sb = load_const("w4", w4[:], [H, NCLS], F16)
            b1_sb = load_const("b1", b1[:], [H, 1], F32)
            b2_sb = load_const("b2", b2[:], [H, 1], F32)
            b3_sb = load_const("b3", b3[:], [H, 1], F32)
            b4_sb = load_const("b4", b4[:], [NCLS, 1], F32)
            idx_sb = load_const("idx", idx_t[:], [128, T * 8], I16)
            rowv_sb = load_const("rowv", rowv_t[:], [128, T], F32)
            vals_sb = load_const("vals", vals_t[:], [128, T], F32)
            iota_sb = load_const("iota", iota_t[:], [128, 128], F16)

            h1_f2 = cpool.tile([128, RP], F16)   # h1 then feat2 (fm) on p0..63
            h_cat = cpool.tile([128, RP], F16)   # feat0 on p0..63, feat1 on p64..127
            f0_rm = cpool.tile([128, NB * H], F32)
            f1_rm = cpool.tile([128, NB * H], F32)
            f2_rm = cpool.tile([128, NB * H], F16)
            f1_16 = cpool.tile([128, NB * H], F16)

            # ---------- MLP1 + MLP2 (feature-major fp16) ----------
            for o in range(0, RP, PCH):
                xt = mpool.tile([F_IN, PCH], F16, tag="xin")
                nc.sync.dma_start(out=xt[:], in_=xT[:, o:o + PCH])
                pt = pspool.tile([H, PCH], F32, tag="pmlp", space="PSUM")
                nc.tensor.matmul(pt[:], lhsT=w1_sb[:], rhs=xt[:],
                                 start=True, stop=True)
                nc.scalar.activation(h1_f2[0:H, o:o + PCH], pt[:],
                                     mybir.ActivationFunctionType.Relu,
                                     bias=b1_sb[:], scale=1.0)
            for o in range(0, RP, PCH):
                pt = pspool.tile([H, PCH], F32, tag="pmlp", space="PSUM")
                nc.tensor.matmul(pt[:], lhsT=w2_sb[:], rhs=h1_f2[0:H, o:o + PCH],
                                 start=True, stop=True)
                nc.scalar.activation(h_cat[0:H, o:o + PCH], pt[:],
                                     mybir.ActivationFunctionType.Relu,
                                     bias=b2_sb[:], scale=1.0)

            # ---------- feat0 -> row-major fp32, ship to AllGather ----------
            for b in range(NB):
                pt = pbpool.tile([128, 128], F16, tag="ptr", space="PSUM")
                nc.tensor.transpose(pt[0:128, 0:H],
                                    h_cat[0:H, b * 128:(b + 1) * 128],
                                    ident[0:H, 0:H])
                nc.vector.tensor_copy(f0_rm[:, b * H:(b + 1) * H], pt[0:128, 0:H])
            nc.sync.dma_start(out=ag_in[0][:], in_=f0_rm[:])
            nc.gpsimd.collective_compute(
                "AllGather", mybir.AluOpType.bypass, replica_groups=rg,
                ins=[ag_in[0][:]], outs=[ag_out[0][:]])

            # ---------- SpMM pass ----------
            qrr = [0]            # gather queue round-robin

            def spmm(src, cur_rm, nxt_rm, nxt_f16, ag_next):
                src_flat = src[:].rearrange("p x -> (p x)")
                qviews = [src_flat[qq * QS * H:(qq + 1) * QS * H].rearrange(
                    "(q s) -> q s", s=H) for qq in range(NQ)]
                for blocks, t0, tcount in groups:
                    gt = gpool.tile([128, gmax * H], F32, tag="gbuf")
                    g16 = gpool.tile([128, gmax * H], F16, tag="g16")
                    goff = 0
                    gslot = {}
                    for qq in range(NQ):
                        ks = [t for t in range(t0, t0 + tcount)
                              if q_of_chunk[t] == qq]
                        if not ks:
                            continue
                        # contiguous qidx run within the group
                        lo = int(qidx_of_chunk[ks[0]])
                        assert int(qidx_of_chunk[ks[-1]]) == lo + len(ks) - 1
                        for s0 in range(0, len(ks), 64):
                            nch = min(64, len(ks) - s0)
                            nc.gpsimd.dma_gather(
                                out_ap=gt[:, goff * H:(goff + nch) * H]
                                .rearrange("p (t e) -> p t e", e=H),
                                in_ap=qviews[qq],
                                idxs_ap=idx_sb[:, qbase[qq] + 8 * (lo + s0):
                                               qbase[qq] + 8 * (lo + s0 + nch)],
                                num_idxs=nch * 128,
                                num_idxs_reg=nch * 128,
                                elem_size=H,
                                single_packet=False,
                                queue_num=qrr[0],
                            )
                            qrr[0] = (qrr[0] + 1) % 4
                            nc.scalar.copy(
                                g16[:, goff * H:(goff + nch) * H],
                                gt[:, goff * H:(goff + nch) * H])
                            for j in range(nch):
                                gslot[ks[s0 + j]] = goff + j
                            goff += nch
                    t = t0
                    for b in blocks:
                        pt = pbpool.tile([128, H], F32, tag="pblk", space="PSUM")
                        nchunks = int(sum(cells[b, qq] for qq in range(NQ)))
                        for k in range(nchunks):
                            gsl = gslot[t]
                            st = spool.tile([128, 128], F16, tag="stile")
                            nc.vector.tensor_scalar(
                                out=st[:], in0=iota_sb[:],
                                scalar1=rowv_sb[:, t:t + 1],
                                scalar2=vals_sb[:, t:t + 1],
                                op0=mybir.AluOpType.is_equal,
                                op1=mybir.AluOpType.mult)
                            nc.tensor.matmul(
                                pt[:],
                                lhsT=st[:],
                                rhs=g16[:, gsl * H:(gsl + 1) * H],
                                start=(k == 0), stop=(k == nchunks - 1))
                            t += 1
                        nc.vector.tensor_tensor(
                            out=(nxt_f16 if nxt_rm is None else nxt_rm)
                            [:, b * H:(b + 1) * H],
                            in0=cur_rm[:, b * H:(b + 1) * H], in1=pt[:],
                            op=mybir.AluOpType.subtract)
                if ag_next is not None:
                    nc.sync.dma_start(out=ag_next[:], in_=nxt_rm[:])

            spmm(ag_out[0], f0_rm, f1_rm, None, ag_in[1])
            nc.gpsimd.collective_compute(
                "AllGather", mybir.AluOpType.bypass, replica_groups=rg,
                ins=[ag_in[1][:]], outs=[ag_out[1][:]])
            spmm(ag_out[1], f1_rm, None, f2_rm, None)

            # ---------- feat1/feat2 back to feature-major ----------
            nc.vector.tensor_copy(f1_16[:], f1_rm[:])
            for b in range(NB):
                pt = pbpool.tile([128, 128], F16, tag="ptr", space="PSUM")
                nc.tensor.transpose(pt[0:H, 0:128], f1_16[:, b * H:(b + 1) * H],
                                    ident[:])
                nc.vector.tensor_copy(h_cat[H:128, b * 128:(b + 1) * 128],
                                      pt[0:H, 0:128])
            for b in range(NB):
                pt = pbpool.tile([128, 128], F16, tag="ptr", space="PSUM")
                nc.tensor.transpose(pt[0:H, 0:128], f2_rm[:, b * H:(b + 1) * H],
                                    ident[:])
                nc.vector.tensor_copy(h1_f2[0:H, b * 128:(b + 1) * 128],
                                      pt[0:H, 0:128])

            # ---------- MLP3 + MLP4 fused ----------
            for o in range(0, RP, PCH):
                pt = pspool.tile([H, PCH], F32, tag="pmlp", space="PSUM")
                nc.tensor.matmul(pt[:], lhsT=w3ab_sb[:], rhs=h_cat[:, o:o + PCH],
                                 start=True, stop=False)
                nc.tensor.matmul(pt[:], lhsT=w3c_sb[:], rhs=h1_f2[0:H, o:o + PCH],
                                 start=False, stop=True)
                h3 = mpool.tile([H, PCH], F16, tag="h3")
                nc.scalar.activation(h3[:], pt[:],
                                     mybir.ActivationFunctionType.Relu,
                                     bias=b3_sb[:], scale=1.0)
                po = pspool.tile([NCLS, PCH], F32, tag="pout", space="PSUM")
                nc.tensor.matmul(po[:], lhsT=w4_sb[:], rhs=h3[:],
                                 start=True, stop=True)
                ot = mpool.tile([NCLS, PCH], F32, tag="ot")
                nc.scalar.activation(ot[:], po[:],
                                     mybir.ActivationFunctionType.Identity,
                                     bias=b4_sb[:], scale=1.0)
                nc.sync.dma_start(out=out_t[:, o:o + PCH], in_=ot[:])

    nc.compile()
    return nc


def _plan(in_feat, adj_rows, adj_cols, adj_vals, W1, b1, W2, b2, W3, b3, W4, b4):
    in_feat = np.asarray(in_feat, dtype=np.float32)
    adj_rows = np.asarray(adj_rows).astype(np.int64)
    adj_cols = np.asarray(adj_cols).astype(np.int64)
    adj_vals = np.asarray(adj_vals, dtype=np.float32)

    thetas = _theta2()
    W3 = np.asarray(W3, dtype=np.float64)
    W3p = np.zeros((3 * H, H), dtype=np.float64)
    for k in range(D + 1):
        for t in range(D + 1):
            W3p[k * H:(k + 1) * H] += thetas[t][k] * W3[t * H:(t + 1) * H]

    (idx_wrapped, rowv, vals, cells, tq, q_of_chunk, qidx_of_chunk
     ) = _prep_edges(adj_rows, adj_cols, adj_vals)

    nc = _build(cells, tq, q_of_chunk, qidx_of_chunk)

    iota = np.tile(np.arange(128, dtype=np.float16), (128, 1))
    in_maps = []
    for c in range(W):
        shard = np.zeros((F_IN, RP), dtype=np.float16)
        shard[:, :R] = in_feat[c * R:(c + 1) * R].T.astype(np.float16)
        in_maps.append({
            "xT": shard,
            "w1": np.asarray(W1).astype(np.float16),
            "w2": np.asarray(W2).astype(np.float16),
            "w3": W3p.astype(np.float16),
            "w4": np.asarray(W4).astype(np.float16),
            "b1": np.asarray(b1, dtype=np.float32).reshape(H, 1),
            "b2": np.asarray(b2, dtype=np.float32).reshape(H, 1),
            "b3": np.asarray(b3, dtype=np.float32).reshape(H, 1),
            "b4": np.asarray(b4, dtype=np.float32).reshape(NCLS, 1),
            "idx": idx_wrapped[c],
            "rowv": rowv[c],
            "vals": vals[c],
            "iota": iota,
        })
    return nc, in_maps


def kernel(in_feat, adj_rows, adj_cols, adj_vals, W1, b1, W2, b2, W3, b3, W4, b4):
    nc, in_maps = _plan(in_feat, adj_rows, adj_cols, adj_vals,
                        W1, b1, W2, b2, W3, b3, W4, b4)
    res = bass_utils.run_bass_kernel_spmd(nc, in_maps, list(range(W)))
    out = np.concatenate(
        [res.results[c]["out"][:, :R].T for c in range(W)], axis=0)
    return np.ascontiguousarray(out, dtype=np.float32)


# revision 9
# speedup vs baseline: 1.7882x; 1.1733x over previous
"""BWGNN (Bernstein-polynomial graph conv, D=2) on 8 Trainium2 NeuronCores.

Key algebra: inside each polyconv the feat sequence f0, f1=f0-A f0, f2=f1-A f1
is theta-independent, so the device needs only TWO SpMMs (not six), and the
Bernstein mixing folds into W3 on the host:
    out = relu([f0|f1|f2] @ W3' + b3) @ W4 + b4,
    W3'[k*H+i, j] = sum_t theta[t][k] * W3[t*H+i, j].

Distribution (8 cores, SPMD single program):
- nodes row-sharded: core c owns rows [c*12500, (c+1)*12500), padded to 12544.
- dense layers stream feature-major fp16 with stationary weights.
- SpMM: edges partitioned by destination; per (128-row dest block b,
  source-column quarter q) cell the edges are padded to whole chunks of 128
  (chunk counts equalized across cores so one program serves all).
- per chunk the PE computes G16.T @ S -> feature-major PSUM [H, 128], where
  G16 holds the gathered+value-scaled source rows and S is the one-hot
  dest-row selector.  Subtract then writes feat_next feature-major straight
  into the MLP3 input tiles (no output transposes).
- S for a whole group is built in ONE DVE tensor_tensor is_equal with
  broadcast APs (iota vs rowv); value scaling + fp32->fp16 convert of the
  gathered rows is ONE DVE tensor_tensor mult per gather batch.
- gathers round-robin over 4 SWDGE queues from an AllGathered fp32 table
  (256B rows; int16 indices are quarter-local, hence 4 quarters).  The table
  is written in SBUF-dump order (one contiguous DMA); indices are
  host-permuted to match.
"""
import math
import numpy as np

import concourse.bass as bass
import concourse.bacc as bacc
import concourse.mybir as mybir
from concourse.tile import TileContext
from concourse.masks import make_identity
from concourse import bass_utils

N = 100000
F_IN = 128
H = 64
NCLS = 2
D = 2
W = 8                   # cores
R = 12500               # real rows per core
RP = 12544              # padded rows per core (98 * 128)
NB = RP // 128          # 98 dest blocks per core
NPAD = W * RP           # 100352 padded table rows
NQ = 4                  # source-column quarters (int16 index range)
QS = NPAD // NQ         # 25088 rows per quarter
GRP = 4                 # dest blocks per gather/S group
F16 = mybir.dt.float16
F32 = mybir.dt.float32
I16 = mybir.dt.int16


def _theta2():
    P = np.polynomial.polynomial
    thetas = []
    for i in range(D + 1):
        beta = math.factorial(i) * math.factorial(D - i) / math.factorial(D + 1)
        c = P.polymul(P.polypow([0.0, 0.5], i), P.polypow([1.0, -0.5], D - i)) / beta
        c = np.pad(c, (0, D + 1 - len(c)))
        thetas.append(c.astype(np.float64))
    return thetas


def _schedule(cells, tq, q_of_chunk, qidx_of_chunk):
    """Device/host-shared schedule: groups of GRP dest blocks; within a group
    the gather stream visits quarters in order, chunks qidx-ascending.

    Returns groups list [(blocks, t0, tcount, runs, slot_base)] where runs =
    [(qq, lo, ks)] per quarter with ks = global chunk ids in stream order, and
    slot_of_t: global chunk id -> global slot index (gather-stream order).
    """
    T = int(tq.sum())
    groups = []
    slot_of_t = np.zeros(T, dtype=np.int64)
    t0 = 0
    slot = 0
    for g0 in range(0, NB, GRP):
        blocks = list(range(g0, min(g0 + GRP, NB)))
        tcount = int(sum(cells[b, qq] for b in blocks for qq in range(NQ)))
        runs = []
        sbase = slot
        for qq in range(NQ):
            ks = [t for t in range(t0, t0 + tcount) if q_of_chunk[t] == qq]
            if not ks:
                continue
            lo = int(qidx_of_chunk[ks[0]])
            assert int(qidx_of_chunk[ks[-1]]) == lo + len(ks) - 1
            runs.append((qq, lo, ks))
            for k in ks:
                slot_of_t[k] = slot
                slot += 1
        groups.append((blocks, t0, tcount, runs, sbase))
        t0 += tcount
    return groups, slot_of_t


def _prep_edges(adj_rows, adj_cols, adj_vals):
    """Partition edges by (core, dest block, source quarter); pad each cell to
    whole 128-edge chunks with counts equalized across cores.

    Table rows live in SBUF-dump order: node (c, rl) -> row
    c*12544 + (rl%128)*NB + rl//128 (in units of H-element rows).

    Returns:
      idx_wrapped[c]: [128, 8*T] int16 gather indices (16-wrapped, 8x replicated)
      rowv[c]: [128, T] fp16, vals[c]: [128, T] fp32 in gather-SLOT order
      cells, tq, q_of_chunk, qidx_of_chunk: shared schedule
    """
    core = adj_rows // R
    rloc = adj_rows - core * R
    blk = rloc // 128
    rowin = rloc % 128
    csrc = adj_cols // R
    rsrc = adj_cols - csrc * R
    colp = csrc * RP + (rsrc % 128) * NB + rsrc // 128   # permuted table row
    q = colp // QS
    qoff = colp - q * QS

    counts = np.zeros((W, NB, NQ), dtype=np.int64)
    np.add.at(counts, (core, blk, q), 1)
    cells = np.maximum(np.ceil(counts.max(axis=0) / 128.0).astype(np.int64), 1)

    tq = cells.sum(axis=0)
    T = int(tq.sum())
    q_of_chunk = np.zeros(T, dtype=np.int64)
    qidx_of_chunk = np.zeros(T, dtype=np.int64)
    qpos = np.zeros(NQ, dtype=np.int64)
    cell_chunk_start = np.zeros((NB, NQ), dtype=np.int64)
    t = 0
    for b in range(NB):
        for qq in range(NQ):
            cell_chunk_start[b, qq] = qpos[qq]
            for _ in range(cells[b, qq]):
                q_of_chunk[t] = qq
                qidx_of_chunk[t] = qpos[qq]
                qpos[qq] += 1
                t += 1

    _, slot_of_t = _schedule(cells, tq, q_of_chunk, qidx_of_chunk)

    order = np.lexsort((q, blk, core))
    sc, sb, sq = core[order], blk[order], q[order]
    s_qoff, s_rowin, s_val = qoff[order], rowin[order], adj_vals[order]

    rowv = np.zeros((W, 128, T), dtype=np.float16)
    vals = np.zeros((W, 128, T), dtype=np.float32)
    idx16 = [[np.zeros(int(tq[qq]) * 128, dtype=np.int16) for qq in range(NQ)]
             for _ in range(W)]

    t_of_q_qidx = np.zeros((NQ, int(tq.max())), dtype=np.int64)
    for tt in range(T):
        t_of_q_qidx[q_of_chunk[tt], qidx_of_chunk[tt]] = tt

    csel = np.searchsorted(sc, np.arange(W + 1))
    for c in range(W):
        lo, hi = csel[c], csel[c + 1]
        b_arr, q_arr = sb[lo:hi], sq[lo:hi]
        qo, ri, vv = s_qoff[lo:hi], s_rowin[lo:hi], s_val[lo:hi]
        m = len(b_arr)
        if not m:
            continue
        key = b_arr * NQ + q_arr
        brk = np.nonzero(np.diff(key))[0] + 1
        starts = np.concatenate([[0], brk])
        lens = np.diff(np.concatenate([starts, [m]]))
        pos = np.arange(m) - np.repeat(starts, lens)
        slot_in_q = cell_chunk_start[b_arr, q_arr] * 128 + pos
        for qq in range(NQ):
            sel = q_arr == qq
            idx16[c][qq][slot_in_q[sel]] = qo[sel].astype(np.int16)
        kchunk = pos // 128
        p = pos % 128
        tglob = t_of_q_qidx[q_arr, cell_chunk_start[b_arr, q_arr] + kchunk]
        sl = slot_of_t[tglob]
        rowv[c, p, sl] = ri.astype(np.float16)
        vals[c, p, sl] = vv.astype(np.float32)

    idx_wrapped = []
    for c in range(W):
        parts = []
        for qq in range(NQ):
            a = idx16[c][qq]
            parts.append(a.reshape(len(a) // 16, 16).T)
        cat = np.concatenate(parts, axis=1)
        idx_wrapped.append(np.tile(cat, (8, 1)).copy())
    return idx_wrapped, rowv, vals, cells, tq, q_of_chunk, qidx_of_chunk


def _build(cells, tq, q_of_chunk, qidx_of_chunk):
    T = int(tq.sum())
    qbase = [8 * int(tq[:qq].sum()) for qq in range(NQ)]
    groups, _ = _schedule(cells, tq, q_of_chunk, qidx_of_chunk)
    gmax = max(tc for _, _, tc, _, _ in groups)

    nc = bacc.Bacc("TRN2", num_swdge_queues=4)
    rg = [list(range(W))]

    xT = nc.dram_tensor("xT", [F_IN, RP], F16, kind="ExternalInput")
    w1 = nc.dram_tensor("w1", [F_IN, H], F16, kind="ExternalInput")
    w2 = nc.dram_tensor("w2", [H, H], F16, kind="ExternalInput")
    w3 = nc.dram_tensor("w3", [3 * H, H], F16, kind="ExternalInput")
    w4 = nc.dram_tensor("w4", [H, NCLS], F16, kind="ExternalInput")
    b1 = nc.dram_tensor("b1", [H, 1], F32, kind="ExternalInput")
    b2 = nc.dram_tensor("b2", [H, 1], F32, kind="ExternalInput")
    b3 = nc.dram_tensor("b3", [H, 1], F32, kind="ExternalInput")
    b4 = nc.dram_tensor("b4", [NCLS, 1], F32, kind="ExternalInput")
    idx_t = nc.dram_tensor("idx", [128, T * 8], I16, kind="ExternalInput")
    rowv_t = nc.dram_tensor("rowv", [128, T], F16, kind="ExternalInput")
    vals_t = nc.dram_tensor("vals", [128, T], F32, kind="ExternalInput")
    iota_t = nc.dram_tensor("iota", [128, 128], F16, kind="ExternalInput")
    out_t = nc.dram_tensor("out", [NCLS, RP], F32, kind="ExternalOutput")

    ag_in = [nc.dram_tensor(f"agin{i}", [128, NB * H], F32, kind="Internal")
             for i in range(2)]
    ag_out = [nc.dram_tensor(f"agout{i}", [W * 128, NB * H], F32,
                             kind="Internal", addr_space="Shared")
              for i in range(2)]

    PCH = 448            # dense-layer column chunk (28 * 448 = 12544)

    with TileContext(nc) as tc:
        with tc.tile_pool(name="c0", bufs=1) as cpool, \
             tc.tile_pool(name="mm", bufs=3) as mpool, \
             tc.tile_pool(name="gg", bufs=2) as gpool, \
             tc.tile_pool(name="sl", bufs=2) as slpool, \
             tc.tile_pool(name="ps", bufs=2, space="PSUM") as pspool, \
             tc.tile_pool(name="pb", bufs=2, space="PSUM") as pbpool:

            ident = cpool.tile([128, 128], F16)
            make_identity(nc, ident[:])

            def load_const(name, src, shape, dt):
                tile = cpool.tile(shape, dt, tag=name)
                nc.sync.dma_start(out=tile[:], in_=src)
                return tile

            w1_sb = load_const("w1", w1[:], [F_IN, H], F16)
            w2_sb = load_const("w2", w2[:], [H, H], F16)
            w3ab_sb = load_const("w3ab", w3[0:128, :], [128, H], F16)
            w3c_sb = load_const("w3c", w3[128:192, :], [H, H], F16)
            w4_sb = load_const("w4", w4[:], [H, NCLS], F16)
            b1_sb = load_const("b1", b1[:], [H, 1], F32)
            b2_sb = load_const("b2", b2[:], [H, 1], F32)
            b3_sb = load_const("b3", b3[:], [H, 1], F32)
            b4_sb = load_const("b4", b4[:], [NCLS, 1], F32)
            idx_sb = load_const("idx", idx_t[:], [128, T * 8], I16)
            rowv_sb = load_const("rowv", rowv_t[:], [128, T], F16)
            vals_sb = load_const("vals", vals_t[:], [128, T], F32)
            iota_sb = load_const("iota", iota_t[:], [128, 128], F16)

            h1_f2 = cpool.tile([128, RP], F16)   # h1 then feat2 (fm) on p0..63
            h_cat = cpool.tile([128, RP], F16)   # feat0 on p0..63, feat1 on p64..127
            f0_rm = cpool.tile([128, NB * H], F32)
            f1_rm = cpool.tile([128, NB * H], F32)

            # ---------- MLP1 + MLP2 (feature-major fp16) ----------
            for o in range(0, RP, PCH):
                xt = mpool.tile([F_IN, PCH], F16, tag="xin")
                nc.sync.dma_start(out=xt[:], in_=xT[:, o:o + PCH])
                pt = pspool.tile([H, PCH], F32, tag="pmlp", space="PSUM")
                nc.tensor.matmul(pt[:], lhsT=w1_sb[:], rhs=xt[:],
                                 start=True, stop=True)
                nc.scalar.activation(h1_f2[0:H, o:o + PCH], pt[:],
                                     mybir.ActivationFunctionType.Relu,
                                     bias=b1_sb[:], scale=1.0)
            for o in range(0, RP, PCH):
                pt = pspool.tile([H, PCH], F32, tag="pmlp", space="PSUM")
                nc.tensor.matmul(pt[:], lhsT=w2_sb[:], rhs=h1_f2[0:H, o:o + PCH],
                                 start=True, stop=True)
                nc.scalar.activation(h_cat[0:H, o:o + PCH], pt[:],
                                     mybir.ActivationFunctionType.Relu,
                                     bias=b2_sb[:], scale=1.0)

            # ---------- feat -> row-major fp32 table, ship to AllGather ------
            def ship(rows, rm, agi, idsl):
                for b in range(NB):
                    pt = pbpool.tile([128, 128], F16, tag="ptr", space="PSUM")
                    nc.tensor.transpose(pt[0:128, 0:H],
                                        rows[:, b * 128:(b + 1) * 128],
                                        idsl)
                    nc.scalar.copy(rm[:, b * H:(b + 1) * H], pt[0:128, 0:H])
                nc.sync.dma_start(out=agi[:], in_=rm[:])

            ship(h_cat[0:H, :], f0_rm, ag_in[0], ident[0:H, 0:H])
            nc.gpsimd.collective_compute(
                "AllGather", mybir.AluOpType.bypass, replica_groups=rg,
                ins=[ag_in[0][:]], outs=[ag_out[0][:]])

            # ---------- SpMM pass ----------
            qrr = [0]            # gather queue round-robin

            def spmm(src, cur_fm, nxt_fm):
                """cur_fm/nxt_fm: [H, RP] views (feature-major fp16)."""
                src_flat = src[:].rearrange("p x -> (p x)")
                qviews = [src_flat[qq * QS * H:(qq + 1) * QS * H].rearrange(
                    "(q s) -> q s", s=H) for qq in range(NQ)]
                for blocks, t0, tcount, runs, sbase in groups:
                    gt = gpool.tile([128, gmax * H], F32, tag="gbuf")
                    g16 = gpool.tile([128, gmax * H], F16, tag="g16")
                    sslab = slpool.tile([128, gmax * 128], F16, tag="sslab")
                    nc.vector.tensor_tensor(
                        out=sslab[:, :tcount * 128]
                        .rearrange("p (t r) -> p t r", r=128),
                        in0=iota_sb[:].unsqueeze(1)
                        .to_broadcast([128, tcount, 128]),
                        in1=rowv_sb[:, sbase:sbase + tcount].unsqueeze(2)
                        .to_broadcast([128, tcount, 128]),
                        op=mybir.AluOpType.is_equal)
                    goff = 0
                    gslot = {}
                    for qq, lo, ks in runs:
                        for s0 in range(0, len(ks), 64):
                            nch = min(64, len(ks) - s0)
                            nc.gpsimd.dma_gather(
                                out_ap=gt[:, goff * H:(goff + nch) * H]
                                .rearrange("p (t e) -> p t e", e=H),
                                in_ap=qviews[qq],
                                idxs_ap=idx_sb[:, qbase[qq] + 8 * (lo + s0):
                                               qbase[qq] + 8 * (lo + s0 + nch)],
                                num_idxs=nch * 128,
                                num_idxs_reg=nch * 128,
                                elem_size=H,
                                single_packet=False,
                                queue_num=qrr[0],
                            )
                            qrr[0] = (qrr[0] + 1) % 4
                            nc.vector.tensor_tensor(
                                out=g16[:, goff * H:(goff + nch) * H]
                                .rearrange("p (t e) -> p t e", e=H),
                                in0=gt[:, goff * H:(goff + nch) * H]
                                .rearrange("p (t e) -> p t e", e=H),
                                in1=vals_sb[:, sbase + goff:sbase + goff + nch]
                                .unsqueeze(2).to_broadcast([128, nch, H]),
                                op=mybir.AluOpType.mult)
                            for j in range(nch):
                                gslot[ks[s0 + j]] = goff + j
                            goff += nch
                    t = t0
                    for b in blocks:
                        pt = pbpool.tile([H, 128], F32, tag="pblk", space="PSUM")
                        nchunks = int(sum(cells[b, qq] for qq in range(NQ)))
                        for k in range(nchunks):
                            gsl = gslot[t]
                            nc.tensor.matmul(
                                pt[:],
                                lhsT=g16[:, gsl * H:(gsl + 1) * H],
                                rhs=sslab[:, gsl * 128:(gsl + 1) * 128],
                                start=(k == 0), stop=(k == nchunks - 1))
                            t += 1
                        nc.vector.tensor_tensor(
                            out=nxt_fm[:, b * 128:(b + 1) * 128],
                            in0=cur_fm[:, b * 128:(b + 1) * 128], in1=pt[:],
                            op=mybir.AluOpType.subtract)

            spmm(ag_out[0], h_cat[0:H, :], h_cat[H:128, :])
            ship(h_cat[H:128, :], f1_rm, ag_in[1], ident[H:128, H:128])
            nc.gpsimd.collective_compute(
                "AllGather", mybir.AluOpType.bypass, replica_groups=rg,
                ins=[ag_in[1][:]], outs=[ag_out[1][:]])
            spmm(ag_out[1], h_cat[H:128, :], h1_f2[0:H, :])

            # ---------- MLP3 + MLP4 fused ----------
            for o in range(0, RP, PCH):
                pt = pspool.tile([H, PCH], F32, tag="pmlp", space="PSUM")
                nc.tensor.matmul(pt[:], lhsT=w3ab_sb[:], rhs=h_cat[:, o:o + PCH],
                                 start=True, stop=False)
                nc.tensor.matmul(pt[:], lhsT=w3c_sb[:], rhs=h1_f2[0:H, o:o + PCH],
                                 start=False, stop=True)
                h3 = mpool.tile([H, PCH], F16, tag="h3")
                nc.scalar.activation(h3[:], pt[:],
                                     mybir.ActivationFunctionType.Relu,
                                     bias=b3_sb[:], scale=1.0)
                po = pspool.tile([NCLS, PCH], F32, tag="pout", space="PSUM")
                nc.tensor.matmul(po[:], lhsT=w4_sb[:], rhs=h3[:],
                                 start=True, stop=True)
                ot = mpool.tile([NCLS, PCH], F32, tag="ot")
                nc.scalar.activation(ot[:], po[:],
                                     mybir.ActivationFunctionType.Identity,
                                     bias=b4_sb[:], scale=1.0)
                nc.sync.dma_start(out=out_t[:, o:o + PCH], in_=ot[:])

    nc.compile()
    return nc


def _plan(in_feat, adj_rows, adj_cols, adj_vals, W1, b1, W2, b2, W3, b3, W4, b4):
    in_feat = np.asarray(in_feat, dtype=np.float32)
    adj_rows = np.asarray(adj_rows).astype(np.int64)
    adj_cols = np.asarray(adj_cols).astype(np.int64)
    adj_vals = np.asarray(adj_vals, dtype=np.float32)

    thetas = _theta2()
    W3 = np.asarray(W3, dtype=np.float64)
    W3p = np.zeros((3 * H, H), dtype=np.float64)
    for k in range(D + 1):
        for t in range(D + 1):
            W3p[k * H:(k + 1) * H] += thetas[t][k] * W3[t * H:(t + 1) * H]

    (idx_wrapped, rowv, vals, cells, tq, q_of_chunk, qidx_of_chunk
     ) = _prep_edges(adj_rows, adj_cols, adj_vals)

    nc = _build(cells, tq, q_of_chunk, qidx_of_chunk)

    iota = np.tile(np.arange(128, dtype=np.float16), (128, 1))
    in_maps = []
    for c in range(W):
        shard = np.zeros((F_IN, RP), dtype=np.float16)
        shard[:, :R] = in_feat[c * R:(c + 1) * R].T.astype(np.float16)
        in_maps.append({
            "xT": shard,
            "w1": np.asarray(W1).astype(np.float16),
            "w2": np.asarray(W2).astype(np.float16),
            "w3": W3p.astype(np.float16),
            "w4": np.asarray(W4).astype(np.float16),
            "b1": np.asarray(b1, dtype=np.float32).reshape(H, 1),
            "b2": np.asarray(b2, dtype=np.float32).reshape(H, 1),
            "b3": np.asarray(b3, dtype=np.float32).reshape(H, 1),
            "b4": np.asarray(b4, dtype=np.float32).reshape(NCLS, 1),
            "idx": idx_wrapped[c],
            "rowv": rowv[c],
            "vals": vals[c],
            "iota": iota,
        })
    return nc, in_maps


def kernel(in_feat, adj_rows, adj_cols, adj_vals, W1, b1, W2, b2, W3, b3, W4, b4):
    nc, in_maps = _plan(in_feat, adj_rows, adj_cols, adj_vals,
                        W1, b1, W2, b2, W3, b3, W4, b4)
    res = bass_utils.run_bass_kernel_spmd(nc, in_maps, list(range(W)))
    out = np.concatenate(
        [res.results[c]["out"][:, :R].T for c in range(W)], axis=0)
    return np.ascontiguousarray(out, dtype=np.float32)
